# revision 2
# baseline (speedup 1.0000x reference)
"""Causal self-attention (B=4, S=4096, D=64, H=4) on 8 TRN2 NeuronCores.

Sharding: the 16 (batch, head) pairs are distributed 2-per-core
(core c -> batch c//2, heads (2*(c%2), 2*(c%2)+1)). Each core runs the
full fused attention for its 2 pairs; no cross-core communication.

The baseline was scalar-engine bound: exp over ~18.9M score elements
per core at 1 elem/cycle/lane on ACT was ~170us of a 219us span. This
version splits the elementwise exp across TWO engines:

  - The PE emits t = A*s + B directly (A = 128/ln2 and the two exact
    bf16 constant contraction columns 16256, -5.5 fold the affine into
    the QK matmul; contraction K=18).
  - ~5/8 of chunks: ACT computes exp(s) = Exp(t*(1/A) - B/A) via the
    activation's free affine (scale + bias AP).
  - ~3/8 of chunks: DVE computes a Schraudolph exp: f32->int16
    tensor_copy rounds+saturates, and int16(A*s+B) IS the bf16 bit
    pattern of ~exp(s) (max rel err ~3% per weight; end-to-end ~1.2e-2
    vs the 2e-2 gate, hardware cast semantics verified: round-to-
    nearest-even, saturating).
  - The causal mask is applied by the PE itself: a -1e9 strictly-upper-
    triangular stationary times identity accumulated onto the diagonal
    128x128 sub-blocks. exp(-big) = 0 on ACT; on DVE the cast saturates
    to -32768 = bf16 -0.0.

Scores stay transposed (key on partitions) so P@V needs no transpose;
the softmax denominator comes from a 17th all-ones column in V; the
division happens on host. PV consumes pt chunk-by-chunk right after
the elementwise, keeping the PE dense with K=128 matmuls (HAM-warm)
and pt pools small. Output: [2 pairs, 17, 4096] f32 rows 0..15 =
unnormalized O.T, row 16 = denominator.
"""

import numpy as np
import ml_dtypes

_B, _S, _D = 4, 4096, 64
_H, _Dh = 4, 16
_NC = 8
_SCALE = 1.0 / np.sqrt(_Dh)
_NQB = _S // 512  # 8 query super-blocks of 512
_NKB = _S // 128  # 32 key blocks of 128
_CHUNK = 3  # k-blocks per elementwise chunk (3 PSUM banks)

_A16 = 128.0 / np.log(2.0)  # Schraudolph slope for bf16 bit pattern
_B16 = 16256.0 - 5.5  # 127*2^7 - c, c=5.5 tuned; = 16256 + (-5.5) exact bf16
_MASKV = -1.0e9

_cache = {}


def _build_nc():
    import concourse.tile as tile
    from concourse import bacc, mybir

    bf = mybir.dt.bfloat16
    f32 = mybir.dt.float32
    i16 = mybir.dt.int16
    Exp = mybir.ActivationFunctionType.Exp

    nc = bacc.Bacc("TRN2", target_bir_lowering=False, debug=False, num_devices=_NC)
    xT_d = nc.dram_tensor("xT", [_D + 1, _S], bf, kind="ExternalInput").ap()
    wqk_d = nc.dram_tensor("wqk", [_D + 1, 100], bf, kind="ExternalInput").ap()
    wv_d = nc.dram_tensor("wv", [_D + 1, 32], bf, kind="ExternalInput").ap()
    mi_d = nc.dram_tensor("mi", [128, 256], bf, kind="ExternalInput").ap()
    out_d = nc.dram_tensor("out", [2, 17, _S], f32, kind="ExternalOutput").ap()

    with tile.TileContext(nc) as tc:
        with tc.tile_pool(name="singles", bufs=1) as singles:
            xT = singles.tile([_D + 1, _S], bf, tag="xT")
            wqk = singles.tile([_D + 1, 100], bf, tag="wqk")
            wv = singles.tile([_D + 1, 32], bf, tag="wv")
            mi = singles.tile([128, 256], bf, tag="mi")
            bias_t = singles.tile([128, 1], f32, tag="bias")
            scratch = singles.tile([128, 1], bf, tag="scratch")
            for c in range(4):
                nc.sync.dma_start(
                    out=xT[:, 1024 * c : 1024 * (c + 1)],
                    in_=xT_d[:, 1024 * c : 1024 * (c + 1)],
                )
            nc.sync.dma_start(out=wqk[:], in_=wqk_d)
            nc.sync.dma_start(out=wv[:], in_=wv_d)
            nc.sync.dma_start(out=mi[:], in_=mi_d)
            nc.vector.memset(bias_t[:], float(-_B16 / _A16))
            # preload the exp table set during the projection phase
            nc.scalar.activation(out=scratch[:], in_=bias_t[:], func=Exp)

            qT = [singles.tile([128, _S], bf, tag=f"qT{p}", name=f"qT{p}") for p in range(2)]
            kT = [singles.tile([128, _S], bf, tag=f"kT{p}", name=f"kT{p}") for p in range(2)]
            V = [singles.tile([128, _NKB, 17], bf, tag=f"V{p}", name=f"V{p}") for p in range(2)]
            for p in range(2):
                nc.vector.memset(V[p][:], 1.0)

            # ---- QKV projections ----
            # wqk per pair p: cols [qe(18) | gap(14) | ke(18)]; qe includes
            # the A-scaling and the two B-constant columns. One DVE copy per
            # chunk into bf16 staging, then gpsimd DMAs replicate to the four
            # 32-aligned partition strips of qT/kT for row-tiled score MMs.
            with (
                tc.tile_pool(name="ps_proj", bufs=3, space="PSUM") as psA,
                tc.tile_pool(name="stgp", bufs=2) as stgp,
            ):
                for p in range(2):
                    stage = None
                    for c in range(_S // 512):
                        csl = slice(512 * c, 512 * (c + 1))
                        pq = psA.tile([50, 512], f32, tag="qk")
                        nc.tensor.matmul(
                            pq[:],
                            wqk[:, 50 * p : 50 * p + 50],
                            xT[:, csl],
                            start=True,
                            stop=True,
                        )
                        if c % 4 == 0:
                            stage = stgp.tile([50, 2048], bf, tag="st", name="st")
                        nc.vector.tensor_copy(
                            stage[:, 512 * (c % 4) : 512 * (c % 4) + 512], pq[:]
                        )
                        if c % 4 == 3:
                            hsl = slice(2048 * (c // 4), 2048 * (c // 4 + 1))
                            for g in range(4):
                                nc.gpsimd.dma_start(
                                    out=qT[p][32 * g : 32 * g + 18, hsl],
                                    in_=stage[0:18, :],
                                )
                                nc.gpsimd.dma_start(
                                    out=kT[p][32 * g : 32 * g + 18, hsl],
                                    in_=stage[32:50, :],
                                )
                for batch in range(4):
                    pv = psA.tile([128, 8, 32], f32, tag="v")
                    for t in range(8):
                        s = 8 * batch + t
                        nc.tensor.matmul(
                            pv[:, t, :],
                            xT[:, 128 * s : 128 * (s + 1)],
                            wv[:],
                            start=True,
                            stop=True,
                        )
                    nc.vector.tensor_copy(
                        V[0][:, 8 * batch : 8 * batch + 8, 0:16], pv[:, :, 0:16]
                    )
                    nc.scalar.copy(
                        V[1][:, 8 * batch : 8 * batch + 8, 0:16], pv[:, :, 16:32]
                    )

            # ---- attention ----
            with (
                tc.tile_pool(name="ps_sc", bufs=2, space="PSUM") as ps_sc,
                tc.tile_pool(name="ps_po", bufs=1, space="PSUM") as ps_po,
                tc.tile_pool(name="ptp", bufs=4) as ptp,
                tc.tile_pool(name="stg", bufs=3) as stg,
            ):
                po = ps_po.tile([128, 512], f32, tag="po", name="po")
                units = [(p, qi) for qi in reversed(range(_NQB)) for p in range(2)]
                ew_count = 0
                for u, (p, qi) in enumerate(units):
                    co = 32 * (u % 4)
                    nkb = 4 * qi + 4
                    qsl = slice(512 * qi, 512 * (qi + 1))
                    nchunks = (nkb + _CHUNK - 1) // _CHUNK
                    for ci in range(nchunks):
                        b0 = ci * _CHUNK
                        nblk = min(_CHUNK, nkb - b0)
                        ps = ps_sc.tile([128, 512 * _CHUNK], f32, tag="sc", name="ps")
                        for t in range(nblk):
                            b = b0 + t
                            g = b % 4
                            j = b - 4 * qi
                            nc.tensor.matmul(
                                ps[:, 512 * t : 512 * (t + 1)],
                                kT[p][32 * g : 32 * g + 18, 128 * b : 128 * (b + 1)],
                                qT[p][32 * g : 32 * g + 18, qsl],
                                start=True,
                                stop=(j < 0),
                                tile_position=(32 * g, 0),
                            )
                            if j >= 0:  # diagonal block: mask via PE accumulate
                                nc.tensor.matmul(
                                    ps[:, 512 * t + 128 * j : 512 * t + 128 * (j + 1)],
                                    mi[:, 0:128],
                                    mi[:, 128:256],
                                    start=False,
                                    stop=True,
                                )
                        pt = ptp.tile([128, 512 * _CHUNK], bf, tag="pt", name="pt")
                        if ew_count % 8 in (2, 5, 7):
                            nc.vector.tensor_copy(
                                pt[:, : 512 * nblk].bitcast(i16),
                                ps[:, : 512 * nblk],
                            )
                        else:
                            nc.scalar.activation(
                                out=pt[:, : 512 * nblk],
                                in_=ps[:, : 512 * nblk],
                                func=Exp,
                                bias=bias_t[:],
                                scale=float(1.0 / _A16),
                            )
                        ew_count += 1
                        for t in range(nblk):
                            b = b0 + t
                            j = b - 4 * qi
                            qoff = 128 * j if j > 0 else 0
                            nc.tensor.matmul(
                                po[co : co + 17, qoff:512],
                                V[p][:, b, :],
                                pt[:, 512 * t + qoff : 512 * (t + 1)],
                                start=(b == 0),
                                stop=(b == nkb - 1),
                                tile_position=(0, co),
                            )
                    ost = stg.tile([17, 512], f32, tag="ost", name="ost")
                    nc.vector.tensor_copy(ost[:], po[co : co + 17, :])
                    nc.sync.dma_start(out=out_d[p][:, qsl], in_=ost[:])

    nc.compile()
    return nc


def _get_nc():
    if "nc" not in _cache:
        _cache["nc"] = _build_nc()
    return _cache["nc"]


def _prepare_in_maps(x, Wq, bq, Wk, bk, Wv, bv):
    bf = ml_dtypes.bfloat16
    x = np.asarray(x, np.float32)
    ones = np.ones((1, _S), np.float32)

    def aug(W, b, h, scale=1.0):
        # [D+1, Dh] block for head h: weight rows plus bias row
        blk = np.concatenate(
            [W[h * _Dh : (h + 1) * _Dh, :], b[h * _Dh : (h + 1) * _Dh, None]], axis=1
        )
        return (blk * scale).T.astype(np.float32)

    # mask|identity: mask[a, b] = -1e9 for b > a (strictly upper), else 0
    mask = np.where(
        np.arange(128)[:, None] < np.arange(128)[None, :], _MASKV, 0.0
    ).astype(np.float32)
    mi = np.concatenate([mask, np.eye(128, dtype=np.float32)], axis=1)

    in_maps = []
    for c in range(_NC):
        b_idx = c // 2
        heads = (2 * (c % 2), 2 * (c % 2) + 1)
        xT = np.concatenate([x[b_idx].T, ones], axis=0)  # [65, 4096]
        wqk_cols = []
        wv_cols = []
        zeros14 = np.zeros((_D + 1, 14), np.float32)
        for h in heads:
            qe = np.zeros((_D + 1, 18), np.float32)
            qe[:, 0:16] = aug(Wq, bq, h, _SCALE * _A16)
            qe[_D, 16] = 16256.0  # with ke col16=1: +16256 to every score
            qe[_D, 17] = -5.5  # with ke col17=1: -5.5 -> t = A*s + 16250.5
            ke = np.zeros((_D + 1, 18), np.float32)
            ke[:, 0:16] = aug(Wk, bk, h)
            ke[_D, 16] = 1.0
            ke[_D, 17] = 1.0
            wqk_cols.extend([qe, zeros14, ke])
            wv_cols.append(aug(Wv, bv, h))
        in_maps.append(
            {
                "xT": xT.astype(bf),
                "wqk": np.concatenate(wqk_cols, axis=1).astype(bf),
                "wv": np.concatenate(wv_cols, axis=1).astype(bf),
                "mi": mi.astype(bf),
            }
        )
    return in_maps


def _assemble(results):
    final = np.empty((_B, _S, _D), np.float32)
    for c in range(_NC):
        b_idx = c // 2
        for p in range(2):
            h = 2 * (c % 2) + p
            o = np.asarray(results[c]["out"], np.float32)  # [2, 17, S]
            final[b_idx, :, h * _Dh : (h + 1) * _Dh] = (o[p, :16] / o[p, 16:17]).T
    return final


def _run(in_maps, trace=False, trace_kwargs=None):
    from concourse.bass_utils import run_bass_kernel_spmd

    nc = _get_nc()
    return run_bass_kernel_spmd(
        nc, in_maps, list(range(_NC)), trace=trace, **(trace_kwargs or {})
    )


def kernel(x, Wq, bq, Wk, bk, Wv, bv):
    in_maps = _prepare_in_maps(x, Wq, bq, Wk, bk, Wv, bv)
    res = _run(in_maps)
    return _assemble(res.results)


# revision 3
# speedup vs baseline: 1.1627x; 1.1627x over previous
"""Causal self-attention (B=4, S=4096, D=64, H=4) on 8 TRN2 NeuronCores.

Sharding: the 16 (batch, head) pairs are distributed 2-per-core
(core c -> batch c//2, heads (2*(c%2), 2*(c%2)+1)). Each core runs the
full fused attention for its 2 pairs; no cross-core communication.

The baseline was scalar-engine bound (exp over ~18.9M score elements at
1 elem/cycle/lane = ~170us). This version splits the elementwise exp
across BOTH PSUM-capable engines:

  - The PE emits t = A*s + B directly (A = 128/ln2; two exact-bf16
    constant contraction columns 16256, -5.5 fold the affine in; K=18).
  - ACT-routed blocks: exp(s) = Exp(t/A - B/A) via the activation's
    free affine (scale + bias AP).
  - DVE-routed blocks: Schraudolph exp = plain f32->int16 tensor_copy;
    int16(A*s+B) IS the bf16 bit pattern of ~exp(s). HW cast semantics
    verified: round-to-nearest-even + saturation, so the -1e9 additive
    causal mask saturates to -32768 = bf16 -0.0 (exact zero weight).
  - Routing is greedy-balanced against a per-engine cost model.
  - The causal mask is a PE matmul accumulate (strictly-upper -1e9
    stationary x identity moving) on diagonal 128x128 sub-blocks only.
  - Masked leading columns of diagonal blocks are never computed: score
    matmuls, elementwise, and PV all start at column 128*j.

The PE runs mostly HAM-throttled at 1.2 GHz (K<32 matmuls count as
idle and the required K=128 duty for warm is ~100%), so PE work is
minimized and parallelized instead: PV runs as 4 concurrent N=128
column-group chains (tile_position col tiling, 8 XBUSes) accumulating
into one zero-initialized PSUM bank per unit (accumulate-only matmuls;
start=True clears has_written for the WHOLE bank - probed - so a K=1
zeroing matmul opens each unit).

Scores stay transposed (key on partitions) so PV needs no transpose;
the softmax denominator comes from a 17th all-ones column in V; the
division happens on host. Output: [2, 17, 4096] f32 = unnormalized O.T
rows 0..15 plus the denominator in row 16.
"""

import numpy as np
import ml_dtypes

_B, _S, _D = 4, 4096, 64
_H, _Dh = 4, 16
_NC = 8
_SCALE = 1.0 / np.sqrt(_Dh)
_NQB = _S // 512
_NKB = _S // 128
_CHUNK = 3

_A16 = 128.0 / np.log(2.0)
_B16 = 16256.0 - 5.5
_MASKV = -1.0e9

_cache = {}


def _build_nc():
    import concourse.tile as tile
    from concourse import bacc, mybir

    bf = mybir.dt.bfloat16
    f32 = mybir.dt.float32
    i16 = mybir.dt.int16
    Exp = mybir.ActivationFunctionType.Exp

    nc = bacc.Bacc("TRN2", target_bir_lowering=False, debug=False, num_devices=_NC)
    xT_d = nc.dram_tensor("xT", [_D + 1, _S], bf, kind="ExternalInput").ap()
    wqk_d = nc.dram_tensor("wqk", [_D + 1, 100], bf, kind="ExternalInput").ap()
    wv_d = nc.dram_tensor("wv", [_D + 1, 32], bf, kind="ExternalInput").ap()
    mi_d = nc.dram_tensor("mi", [128, 256], bf, kind="ExternalInput").ap()
    out_d = nc.dram_tensor("out", [2, 17, _S], f32, kind="ExternalOutput").ap()

    # greedy engine balance: modeled ns accumulated per engine
    ew = {"act": 2700.0, "dve": 1000.0}

    def route(n_elems):
        a = ew["act"] + (n_elems + 352) / 1.2
        d = ew["dve"] + (n_elems + 152) / 0.96
        if d <= a:
            ew["dve"] = d
            return "dve"
        ew["act"] = a
        return "act"

    with tile.TileContext(nc) as tc:
        with tc.tile_pool(name="singles", bufs=1) as singles:
            xT = singles.tile([_D + 1, _S], bf, tag="xT")
            wqk = singles.tile([_D + 1, 100], bf, tag="wqk")
            wv = singles.tile([_D + 1, 32], bf, tag="wv")
            mi = singles.tile([128, 256], bf, tag="mi")
            bias_t = singles.tile([128, 1], f32, tag="bias")
            scratch = singles.tile([128, 1], bf, tag="scratch")
            zrow = singles.tile([1, 640], bf, tag="zrow")
            for c in range(4):
                nc.sync.dma_start(
                    out=xT[:, 1024 * c : 1024 * (c + 1)],
                    in_=xT_d[:, 1024 * c : 1024 * (c + 1)],
                )
            nc.sync.dma_start(out=wqk[:], in_=wqk_d)
            nc.sync.dma_start(out=wv[:], in_=wv_d)
            nc.sync.dma_start(out=mi[:], in_=mi_d)
            nc.vector.memset(bias_t[:], float(-_B16 / _A16))
            nc.vector.memset(zrow[:], 0.0)
            # preload the exp table set during the projection phase
            nc.scalar.activation(out=scratch[:], in_=bias_t[:], func=Exp)

            qT = [singles.tile([128, _S], bf, tag=f"qT{p}", name=f"qT{p}") for p in range(2)]
            kT = [singles.tile([128, _S], bf, tag=f"kT{p}", name=f"kT{p}") for p in range(2)]
            V = [singles.tile([128, _NKB, 17], bf, tag=f"V{p}", name=f"V{p}") for p in range(2)]
            for p in range(2):
                nc.vector.memset(V[p][:], 1.0)

            # ---- QKV projections ----
            with (
                tc.tile_pool(name="ps_proj", bufs=3, space="PSUM") as psA,
                tc.tile_pool(name="stgp", bufs=2) as stgp,
            ):
                for p in range(2):
                    stage = None
                    for c in range(_S // 512):
                        csl = slice(512 * c, 512 * (c + 1))
                        pq = psA.tile([50, 512], f32, tag="qk")
                        nc.tensor.matmul(
                            pq[:],
                            wqk[:, 50 * p : 50 * p + 50],
                            xT[:, csl],
                            start=True,
                            stop=True,
                        )
                        if c % 4 == 0:
                            stage = stgp.tile([50, 2048], bf, tag="st", name="st")
                        nc.vector.tensor_copy(
                            stage[:, 512 * (c % 4) : 512 * (c % 4) + 512], pq[:]
                        )
                        ew["dve"] += (512 + 152) / 0.96
                        if c % 4 == 3:
                            hsl = slice(2048 * (c // 4), 2048 * (c // 4 + 1))
                            for g in range(4):
                                nc.gpsimd.dma_start(
                                    out=qT[p][32 * g : 32 * g + 18, hsl],
                                    in_=stage[0:18, :],
                                )
                                nc.gpsimd.dma_start(
                                    out=kT[p][32 * g : 32 * g + 18, hsl],
                                    in_=stage[32:50, :],
                                )
                for batch in range(4):
                    pv = psA.tile([128, 8, 32], f32, tag="v")
                    for t in range(8):
                        s = 8 * batch + t
                        nc.tensor.matmul(
                            pv[:, t, :],
                            xT[:, 128 * s : 128 * (s + 1)],
                            wv[:],
                            start=True,
                            stop=True,
                        )
                    nc.vector.tensor_copy(
                        V[0][:, 8 * batch : 8 * batch + 8, 0:16], pv[:, :, 0:16]
                    )
                    nc.scalar.copy(
                        V[1][:, 8 * batch : 8 * batch + 8, 0:16], pv[:, :, 16:32]
                    )
                    ew["dve"] += (128 + 152) / 0.96
                    ew["act"] += (128 + 352) / 1.2

            # ---- attention ----
            with (
                tc.tile_pool(name="ps_sc", bufs=2, space="PSUM") as ps_sc,
                tc.tile_pool(name="ps_po", bufs=2, space="PSUM") as ps_po,
                tc.tile_pool(name="ptp", bufs=4) as ptp,
                tc.tile_pool(name="stg", bufs=3) as stg,
            ):
                units = [(p, qi) for qi in reversed(range(_NQB)) for p in range(2)]
                for p, qi in units:
                    nkb = 4 * qi + 4
                    q0 = 512 * qi
                    nchunks = (nkb + _CHUNK - 1) // _CHUNK
                    po = ps_po.tile([128, 512], f32, tag="po", name="po")
                    # zero the bank: K=1 matmul of zeros, start=True clears
                    # has_written bank-wide; chains below accumulate-only
                    nc.tensor.matmul(
                        po[:],
                        zrow[0:1, 0:128],
                        zrow[0:1, 128:640],
                        start=True,
                        stop=False,
                    )
                    for ci in range(nchunks):
                        b0 = ci * _CHUNK
                        nblk = min(_CHUNK, nkb - b0)
                        ps = ps_sc.tile([128, 512 * _CHUNK], f32, tag="sc", name="ps")
                        for t in range(nblk):
                            b = b0 + t
                            g = b % 4
                            j = b - 4 * qi
                            off = 128 * j if j > 0 else 0
                            nc.tensor.matmul(
                                ps[:, 512 * t + off : 512 * (t + 1)],
                                kT[p][32 * g : 32 * g + 18, 128 * b : 128 * (b + 1)],
                                qT[p][32 * g : 32 * g + 18, q0 + off : q0 + 512],
                                start=True,
                                stop=(j < 0),
                                tile_position=(32 * g, 0),
                            )
                            if j >= 0:  # diagonal: accumulate -1e9 upper-tri
                                nc.tensor.matmul(
                                    ps[:, 512 * t + 128 * j : 512 * t + 128 * (j + 1)],
                                    mi[:, 0:128],
                                    mi[:, 128:256],
                                    start=False,
                                    stop=True,
                                )
                        pt = ptp.tile([128, 512 * _CHUNK], bf, tag="pt", name="pt")
                        # elementwise spans: normal prefix as one instr, then
                        # per-block trimmed instrs for diagonal blocks
                        spans = []
                        ndiag0 = next(
                            (t for t in range(nblk) if b0 + t - 4 * qi >= 0), nblk
                        )
                        if ndiag0 > 0:
                            spans.append((0, 512 * ndiag0))
                        for t in range(ndiag0, nblk):
                            j = b0 + t - 4 * qi
                            spans.append((512 * t + 128 * j, 512 * (t + 1)))
                        for lo, hi in spans:
                            if route(hi - lo) == "dve":
                                nc.vector.tensor_copy(
                                    pt[:, lo:hi].bitcast(i16), ps[:, lo:hi]
                                )
                            else:
                                nc.scalar.activation(
                                    out=pt[:, lo:hi],
                                    in_=ps[:, lo:hi],
                                    func=Exp,
                                    bias=bias_t[:],
                                    scale=float(1.0 / _A16),
                                )
                        # PV: 4 concurrent col-group chains, one per 128-query
                        # column strip; chain cg takes blocks with j <= cg
                        for t in range(nblk):
                            b = b0 + t
                            j = b - 4 * qi
                            for cg in range(4):
                                if j > cg:
                                    continue
                                nc.tensor.matmul(
                                    po[32 * cg : 32 * cg + 17, 128 * cg : 128 * (cg + 1)],
                                    V[p][:, b, :],
                                    pt[:, 512 * t + 128 * cg : 512 * t + 128 * (cg + 1)],
                                    start=False,
                                    stop=(b == 4 * qi + cg),
                                    tile_position=(0, 32 * cg),
                                )
                    ost = stg.tile([128, 512], f32, tag="ost", name="ost")
                    if route(512) == "dve":
                        nc.vector.tensor_copy(ost[:], po[:])
                    else:
                        nc.scalar.copy(ost[:], po[:])
                    for cg in range(4):
                        nc.sync.dma_start(
                            out=out_d[p][:, q0 + 128 * cg : q0 + 128 * (cg + 1)],
                            in_=ost[32 * cg : 32 * cg + 17, 128 * cg : 128 * (cg + 1)],
                        )

    nc.compile()
    return nc


def _get_nc():
    if "nc" not in _cache:
        _cache["nc"] = _build_nc()
    return _cache["nc"]


def _prepare_in_maps(x, Wq, bq, Wk, bk, Wv, bv):
    bf = ml_dtypes.bfloat16
    x = np.asarray(x, np.float32)
    ones = np.ones((1, _S), np.float32)

    def aug(W, b, h, scale=1.0):
        blk = np.concatenate(
            [W[h * _Dh : (h + 1) * _Dh, :], b[h * _Dh : (h + 1) * _Dh, None]], axis=1
        )
        return (blk * scale).T.astype(np.float32)

    # mask|identity: mask[a, b] = -1e9 for b > a (strictly upper), else 0
    mask = np.where(
        np.arange(128)[:, None] < np.arange(128)[None, :], _MASKV, 0.0
    ).astype(np.float32)
    mi = np.concatenate([mask, np.eye(128, dtype=np.float32)], axis=1)

    in_maps = []
    for c in range(_NC):
        b_idx = c // 2
        heads = (2 * (c % 2), 2 * (c % 2) + 1)
        xT = np.concatenate([x[b_idx].T, ones], axis=0)
        wqk_cols = []
        wv_cols = []
        zeros14 = np.zeros((_D + 1, 14), np.float32)
        for h in heads:
            qe = np.zeros((_D + 1, 18), np.float32)
            qe[:, 0:16] = aug(Wq, bq, h, _SCALE * _A16)
            qe[_D, 16] = 16256.0
            qe[_D, 17] = -5.5
            ke = np.zeros((_D + 1, 18), np.float32)
            ke[:, 0:16] = aug(Wk, bk, h)
            ke[_D, 16] = 1.0
            ke[_D, 17] = 1.0
            wqk_cols.extend([qe, zeros14, ke])
            wv_cols.append(aug(Wv, bv, h))
        in_maps.append(
            {
                "xT": xT.astype(bf),
                "wqk": np.concatenate(wqk_cols, axis=1).astype(bf),
                "wv": np.concatenate(wv_cols, axis=1).astype(bf),
                "mi": mi.astype(bf),
            }
        )
    return in_maps


def _assemble(results):
    final = np.empty((_B, _S, _D), np.float32)
    for c in range(_NC):
        b_idx = c // 2
        for p in range(2):
            h = 2 * (c % 2) + p
            o = np.asarray(results[c]["out"], np.float32)
            final[b_idx, :, h * _Dh : (h + 1) * _Dh] = (o[p, :16] / o[p, 16:17]).T
    return final


def _run(in_maps, trace=False, trace_kwargs=None):
    from concourse.bass_utils import run_bass_kernel_spmd

    nc = _get_nc()
    return run_bass_kernel_spmd(
        nc, in_maps, list(range(_NC)), trace=trace, **(trace_kwargs or {})
    )


def kernel(x, Wq, bq, Wk, bk, Wv, bv):
    in_maps = _prepare_in_maps(x, Wq, bq, Wk, bk, Wv, bv)
    res = _run(in_maps)
    return _assemble(res.results)


# revision 7
# speedup vs baseline: 1.1781x; 1.0133x over previous
"""Causal self-attention (B=4, S=4096, D=64, H=4) on 8 TRN2 NeuronCores.

Sharding: the 16 (batch, head) pairs are distributed 2-per-core
(core c -> batch c//2, heads (2*(c%2), 2*(c%2)+1)). Each core runs the
full fused attention for its 2 pairs; no cross-core communication.

The baseline was scalar-engine bound (exp over ~18.9M score elements at
1 elem/cycle/lane = ~170us). This version splits the elementwise exp
across BOTH PSUM-capable engines:

  - The PE emits t = A*s + B directly (A = 128/ln2; two exact-bf16
    constant contraction columns 16256, -5.5 fold the affine in; K=18).
  - ACT-routed blocks: exp(s) = Exp(t/A - B/A) via the activation's
    free affine (scale + bias AP).
  - DVE-routed blocks: Schraudolph exp = plain f32->int16 tensor_copy;
    int16(A*s+B) IS the bf16 bit pattern of ~exp(s). HW cast semantics
    verified: round-to-nearest-even + saturation, so the -1e9 additive
    causal mask saturates to -32768 = bf16 -0.0 (exact zero weight).
  - Routing is greedy-balanced against a per-engine cost model.

PE structure (HAM keeps the PE at 1.2 GHz for this instruction mix -
warm needs ~100% K=128 duty - so PE work is minimized and parallelized
for the cold clock):
  - scores: 4-way row-tiled K=18 matmuls (4 strips stream ~2.7 cols/ns)
  - causal mask: 4-way row-tiled K=32 accumulate matmuls (strictly-
    upper -1e9 stationary x identity selector rows)
  - PV: 4 concurrent N=128 column-group chains accumulating into one
    PSUM bank per unit. start=True clears has_written BANK-WIDE
    (probed), so only chain 0's first matmul carries start=True and the
    other chains' first writes overwrite-on-cleared-bits. PV batches
    are deferred and emitted every other chunk to reduce class
    switches (scores/PV conflict on PE sub-arrays).
  - masked leading columns of diagonal blocks are never computed.
  - units run ascending qi so the first units only need the first
    replicated half of qT/kT; projections and V are interleaved to
    unblock them early.

Scores stay transposed (key on partitions) so PV needs no transpose;
the softmax denominator comes from a 17th all-ones column in V; the
division happens on host. Output: [2, 17, 4096] f32 = unnormalized O.T
rows 0..15 plus the denominator in row 16.
"""

import numpy as np
import ml_dtypes

_B, _S, _D = 4, 4096, 64
_H, _Dh = 4, 16
_NC = 8
_SCALE = 1.0 / np.sqrt(_Dh)
_NQB = _S // 512
_NKB = _S // 128
_CHUNK = 3

_A16 = 128.0 / np.log(2.0)
_B16 = 16256.0 - 5.5
_MASKV = -1.0e9

_cache = {}


def _build_nc():
    import concourse.tile as tile
    from concourse import bacc, mybir

    bf = mybir.dt.bfloat16
    f32 = mybir.dt.float32
    i16 = mybir.dt.int16
    Exp = mybir.ActivationFunctionType.Exp

    nc = bacc.Bacc("TRN2", target_bir_lowering=False, debug=False, num_devices=_NC)
    xT_d = nc.dram_tensor("xT", [_D + 1, _S], bf, kind="ExternalInput").ap()
    wqk_d = nc.dram_tensor("wqk", [_D + 1, 100], bf, kind="ExternalInput").ap()
    wv_d = nc.dram_tensor("wv", [_D + 1, 32], bf, kind="ExternalInput").ap()
    mi_d = nc.dram_tensor("mi", [128, 256], bf, kind="ExternalInput").ap()
    out_d = nc.dram_tensor("out", [2, 17, _S], f32, kind="ExternalOutput").ap()

    ew = {"act": 2700.0, "dve": 1000.0}

    def route(n_elems):
        a = ew["act"] + (n_elems + 352) / 1.2
        d = ew["dve"] + (n_elems + 152) / 0.96
        if d <= a:
            ew["dve"] = d
            return "dve"
        ew["act"] = a
        return "act"

    with tile.TileContext(nc) as tc:
        with tc.tile_pool(name="singles", bufs=1) as singles:
            xT = singles.tile([_D + 1, _S], bf, tag="xT")
            wqk = singles.tile([_D + 1, 100], bf, tag="wqk")
            wv = singles.tile([_D + 1, 32], bf, tag="wv")
            mi = singles.tile([128, 256], bf, tag="mi")
            bias_t = singles.tile([128, 1], f32, tag="bias")
            scratch = singles.tile([128, 1], bf, tag="scratch")
            zrow = singles.tile([1, 640], bf, tag="zrow")
            nc.vector.memset(zrow[:], 0.0)
            for c in range(4):
                nc.sync.dma_start(
                    out=xT[:, 1024 * c : 1024 * (c + 1)],
                    in_=xT_d[:, 1024 * c : 1024 * (c + 1)],
                )
            nc.sync.dma_start(out=wqk[:], in_=wqk_d)
            nc.sync.dma_start(out=wv[:], in_=wv_d)
            nc.sync.dma_start(out=mi[:], in_=mi_d)
            nc.vector.memset(bias_t[:], float(-_B16 / _A16))
            nc.scalar.activation(out=scratch[:], in_=bias_t[:], func=Exp)

            qT = [singles.tile([128, _S], bf, tag=f"qT{p}", name=f"qT{p}") for p in range(2)]
            kT = [singles.tile([128, _S], bf, tag=f"kT{p}", name=f"kT{p}") for p in range(2)]
            V = [singles.tile([128, _NKB, 17], bf, tag=f"V{p}", name=f"V{p}") for p in range(2)]
            for p in range(2):
                nc.vector.memset(V[p][:], 1.0)

            # ---- QKV projections, interleaved for early attention start ----
            with (
                tc.tile_pool(name="ps_proj", bufs=3, space="PSUM") as psA,
                tc.tile_pool(name="stgp", bufs=2) as stgp,
            ):
                def proj_half(p, half):
                    stage = stgp.tile([50, 2048], bf, tag="st", name="st")
                    for cc in range(4):
                        c = 4 * half + cc
                        csl = slice(512 * c, 512 * (c + 1))
                        pq = psA.tile([50, 512], f32, tag="qk")
                        nc.tensor.matmul(
                            pq[:],
                            wqk[:, 50 * p : 50 * p + 50],
                            xT[:, csl],
                            start=True,
                            stop=True,
                        )
                        nc.vector.tensor_copy(
                            stage[:, 512 * cc : 512 * cc + 512], pq[:]
                        )
                        ew["dve"] += (512 + 152) / 0.96
                    hsl = slice(2048 * half, 2048 * (half + 1))
                    for g in range(4):
                        nc.gpsimd.dma_start(
                            out=qT[p][32 * g : 32 * g + 18, hsl], in_=stage[0:18, :]
                        )
                        nc.gpsimd.dma_start(
                            out=kT[p][32 * g : 32 * g + 18, hsl], in_=stage[32:50, :]
                        )

                def proj_v(batch):
                    pv = psA.tile([128, 8, 32], f32, tag="v")
                    for t in range(8):
                        s = 8 * batch + t
                        nc.tensor.matmul(
                            pv[:, t, :],
                            xT[:, 128 * s : 128 * (s + 1)],
                            wv[:],
                            start=True,
                            stop=True,
                        )
                    nc.vector.tensor_copy(
                        V[0][:, 8 * batch : 8 * batch + 8, 0:16], pv[:, :, 0:16]
                    )
                    nc.scalar.copy(
                        V[1][:, 8 * batch : 8 * batch + 8, 0:16], pv[:, :, 16:32]
                    )
                    ew["dve"] += (128 + 152) / 0.96
                    ew["act"] += (128 + 352) / 1.2

                proj_half(0, 0)
                proj_v(0)
                proj_v(1)
                proj_half(1, 0)
                proj_half(0, 1)
                proj_v(2)
                proj_v(3)
                proj_half(1, 1)

            # ---- attention ----
            with (
                tc.tile_pool(name="ps_sc", bufs=2, space="PSUM") as ps_sc,
                tc.tile_pool(name="ps_po", bufs=2, space="PSUM") as ps_po,
                tc.tile_pool(name="ptp", bufs=5) as ptp,
                tc.tile_pool(name="stg", bufs=3) as stg,
            ):
                units = [(p, qi) for qi in range(_NQB) for p in range(2)]
                for p, qi in units:
                    nkb = 4 * qi + 4
                    q0 = 512 * qi
                    nchunks = (nkb + _CHUNK - 1) // _CHUNK
                    po = ps_po.tile([128, 512], f32, tag="po", name="po")
                    # zero the bank (start=True clears has_written per
                    # partition, and only for written elements - probed);
                    # the K=1 zero matmul covers all 128 partitions x 512
                    # cols so the chains below can be accumulate-only
                    nc.tensor.matmul(
                        po[:],
                        zrow[0:1, 0:128],
                        zrow[0:1, 128:640],
                        start=True,
                        stop=False,
                    )
                    pv_pend = []

                    def flush_pv():
                        for b, t, pt in pv_pend:
                            j = b - 4 * qi
                            for cg in range(4):
                                if j > cg:
                                    continue
                                nc.tensor.matmul(
                                    po[32 * cg : 32 * cg + 17, 128 * cg : 128 * (cg + 1)],
                                    V[p][:, b, :],
                                    pt[:, 512 * t + 128 * cg : 512 * t + 128 * (cg + 1)],
                                    start=False,
                                    stop=(b == 4 * qi + cg),
                                    tile_position=(0, 32 * cg),
                                )
                        pv_pend.clear()

                    for ci in range(nchunks):
                        b0 = ci * _CHUNK
                        nblk = min(_CHUNK, nkb - b0)
                        ps = ps_sc.tile([128, 512 * _CHUNK], f32, tag="sc", name="ps")
                        for t in range(nblk):
                            b = b0 + t
                            g = b % 4
                            j = b - 4 * qi
                            off = 128 * j if j > 0 else 0
                            nc.tensor.matmul(
                                ps[:, 512 * t + off : 512 * (t + 1)],
                                kT[p][32 * g : 32 * g + 18, 128 * b : 128 * (b + 1)],
                                qT[p][32 * g : 32 * g + 18, q0 + off : q0 + 512],
                                start=True,
                                stop=(j < 0),
                                tile_position=(32 * g, 0),
                            )
                            if j >= 0:  # diagonal: accumulate -1e9 upper-tri
                                nc.tensor.matmul(
                                    ps[:, 512 * t + 128 * j : 512 * t + 128 * (j + 1)],
                                    mi[:, 0:128],
                                    mi[:, 128:256],
                                    start=False,
                                    stop=True,
                                )
                        pt = ptp.tile([128, 512 * _CHUNK], bf, tag="pt", name="pt")
                        spans = []
                        ndiag0 = next(
                            (t for t in range(nblk) if b0 + t - 4 * qi >= 0), nblk
                        )
                        if ndiag0 > 0:
                            spans.append((0, 512 * ndiag0))
                        for t in range(ndiag0, nblk):
                            j = b0 + t - 4 * qi
                            spans.append((512 * t + 128 * j, 512 * (t + 1)))
                        for lo, hi in spans:
                            if route(hi - lo) == "dve":
                                nc.vector.tensor_copy(
                                    pt[:, lo:hi].bitcast(i16), ps[:, lo:hi]
                                )
                            else:
                                nc.scalar.activation(
                                    out=pt[:, lo:hi],
                                    in_=ps[:, lo:hi],
                                    func=Exp,
                                    bias=bias_t[:],
                                    scale=float(1.0 / _A16),
                                )
                        for t in range(nblk):
                            pv_pend.append((b0 + t, t, pt))
                        if ci % 2 == 1:
                            flush_pv()
                    flush_pv()
                    ost = stg.tile([128, 512], f32, tag="ost", name="ost")
                    if route(512) == "dve":
                        nc.vector.tensor_copy(ost[:], po[:])
                    else:
                        nc.scalar.copy(ost[:], po[:])
                    for cg in range(4):
                        nc.sync.dma_start(
                            out=out_d[p][:, q0 + 128 * cg : q0 + 128 * (cg + 1)],
                            in_=ost[32 * cg : 32 * cg + 17, 128 * cg : 128 * (cg + 1)],
                        )

    nc.compile()
    return nc


def _get_nc():
    if "nc" not in _cache:
        _cache["nc"] = _build_nc()
    return _cache["nc"]


def _prepare_in_maps(x, Wq, bq, Wk, bk, Wv, bv):
    bf = ml_dtypes.bfloat16
    x = np.asarray(x, np.float32)
    ones = np.ones((1, _S), np.float32)

    def aug(W, b, h, scale=1.0):
        blk = np.concatenate(
            [W[h * _Dh : (h + 1) * _Dh, :], b[h * _Dh : (h + 1) * _Dh, None]], axis=1
        )
        return (blk * scale).T.astype(np.float32)

    mask = np.where(
        np.arange(128)[:, None] < np.arange(128)[None, :], _MASKV, 0.0
    ).astype(np.float32)
    mi = np.concatenate([mask, np.eye(128, dtype=np.float32)], axis=1)

    in_maps = []
    for c in range(_NC):
        b_idx = c // 2
        heads = (2 * (c % 2), 2 * (c % 2) + 1)
        xT = np.concatenate([x[b_idx].T, ones], axis=0)
        wqk_cols = []
        wv_cols = []
        zeros14 = np.zeros((_D + 1, 14), np.float32)
        for h in heads:
            qe = np.zeros((_D + 1, 18), np.float32)
            qe[:, 0:16] = aug(Wq, bq, h, _SCALE * _A16)
            qe[_D, 16] = 16256.0
            qe[_D, 17] = -5.5
            ke = np.zeros((_D + 1, 18), np.float32)
            ke[:, 0:16] = aug(Wk, bk, h)
            ke[_D, 16] = 1.0
            ke[_D, 17] = 1.0
            wqk_cols.extend([qe, zeros14, ke])
            wv_cols.append(aug(Wv, bv, h))
        in_maps.append(
            {
                "xT": xT.astype(bf),
                "wqk": np.concatenate(wqk_cols, axis=1).astype(bf),
                "wv": np.concatenate(wv_cols, axis=1).astype(bf),
                "mi": mi.astype(bf),
            }
        )
    return in_maps


def _assemble(results):
    final = np.empty((_B, _S, _D), np.float32)
    for c in range(_NC):
        b_idx = c // 2
        for p in range(2):
            h = 2 * (c % 2) + p
            o = np.asarray(results[c]["out"], np.float32)
            final[b_idx, :, h * _Dh : (h + 1) * _Dh] = (o[p, :16] / o[p, 16:17]).T
    return final


def _run(in_maps, trace=False, trace_kwargs=None):
    from concourse.bass_utils import run_bass_kernel_spmd

    nc = _get_nc()
    return run_bass_kernel_spmd(
        nc, in_maps, list(range(_NC)), trace=trace, **(trace_kwargs or {})
    )


def kernel(x, Wq, bq, Wk, bk, Wv, bv):
    in_maps = _prepare_in_maps(x, Wq, bq, Wk, bk, Wv, bv)
    res = _run(in_maps)
    return _assemble(res.results)
